# revision 13
# baseline (speedup 1.0000x reference)
"""CopyDecoder Trainium2 kernel (nn_CopyDecoder_5274219840242).

Sharding: 8 cores = 4 batches x 2 query-halves (data parallel, no collectives).

Per core (b, q-slab of 256 rows):
  - attention: cq = fcQ(dec), Q/K projections (computed transposed so the
    contraction dim lands on partitions; bf16 operands, fp32 accumulate),
    per-head softmax (logits bounded, so no max-subtraction), head mean.
  - duplicate-combining selection matrix Dm[s,s'] = [src_s == src_s'] built by
    compare-vs-transpose; a_comb = attn @ Dm gives each source position the
    full scatter-sum of its token; e = exp(a_comb/NH).
  - denom[q] = V + sum_s (e[q,s]-1)/cnt[s]  (softmax denominator over vocab,
    exploiting exp(0)=1 for vocab entries no source token maps to).
  - streaming blend over p1 in BF16 both directions (tolerance is 2e-2):
    out = (1-w)*p1 + w/denom.  Halves HBM traffic vs fp32 streaming, which
    is the roofline here (~99% DMA active in the fp32 baseline trace).
    The fused DVE tensor_scalar double-rounds (bf16 intermediate), which
    costs ~1.1e-2 rel err; instead each tile takes a single-rounding path:
    either one scalar-engine activation (out = Identity(p1*s1 + s2), fp32
    internal, per-partition scale/bias APs) or a DVE pair (mul to fp32
    intermediate, add to bf16).  Tiles are split between the two engines
    so neither becomes the bottleneck.
  - fix values for the <=512 source-token columns:
    fix[q,s] = (1-w)*p1[q,src_s] + (w/denom)*e[q,s]
    (p1 columns are host-gathered fp32 into an extra input; the host writes
    the fix columns into the final output during unshard).

Queue split (two HWDGE rings share 16 DMA engines, ~23.5GB/s each busy):
  - sync ring: pure p1 bf16 load stream, starts at instruction 0.
  - scalar ring: small packed-constants load, weights, p1c, then all
    out-stores + fixc.
The attention chain runs per q-partition-tile (mi) so the first blend
stores start while the second tile's softmax is still in flight.
"""

import sys

sys.path.insert(0, "/opt/trn_rl_repo")

import numpy as np

import concourse.bacc as bacc
import concourse.bass as bass
import concourse.mybir as mybir
import concourse.tile as tile
from concourse.bass_utils import run_bass_kernel_spmd
from concourse.masks import make_identity

P = 128
D = 512
TS = 512
TQH = 256  # q rows per core
V = 32000
NH = 8
DH = 64
KC = D // P  # 4 contraction chunks
MI = TQH // P  # 2 q partition tiles
SC = TS // P  # 4 source-position chunks
VT = 4000  # vocab columns per blend tile (8000B bf16 per partition row)
NVT = V // VT  # 8 vocab tiles per q partition tile

F32 = mybir.dt.float32
BF16 = mybir.dt.bfloat16
I32 = mybir.dt.int32
AF = mybir.ActivationFunctionType
ALU = mybir.AluOpType
AX = mybir.AxisListType

# packed per-partition constants layout (f32 columns):
#   [0:4) srcf   [4:8) bfcq   [8:12) bq   [12:16) bk   [16:20) wfcw  [20] bfcw
PK = 21

_NC_CACHE = None
_LAST_RESULTS = None


def build_nc():
    nc = bacc.Bacc("TRN2", target_bir_lowering=False, debug=False)

    decT = nc.dram_tensor("decT", [D, TQH], F32, kind="ExternalInput")
    decTb = nc.dram_tensor("decTb", [D, TQH], BF16, kind="ExternalInput")
    encTb = nc.dram_tensor("encTb", [D, TS], BF16, kind="ExternalInput")
    wfcqb = nc.dram_tensor("wfcqb", [D, D], BF16, kind="ExternalInput")
    wqb = nc.dram_tensor("wqb", [D, D], BF16, kind="ExternalInput")
    wkb = nc.dram_tensor("wkb", [D, D], BF16, kind="ExternalInput")
    pk = nc.dram_tensor("pk", [P, PK], F32, kind="ExternalInput")
    p1 = nc.dram_tensor("p1", [TQH, V], BF16, kind="ExternalInput")
    p1c = nc.dram_tensor("p1c", [TQH, TS], BF16, kind="ExternalInput")
    out = nc.dram_tensor("out", [TQH, V], BF16, kind="ExternalOutput")
    fixc = nc.dram_tensor("fixc", [TQH, TS], F32, kind="ExternalOutput")

    with tile.TileContext(nc) as tc:
        with (
            tc.tile_pool(name="const", bufs=1) as cp,
            tc.tile_pool(name="work", bufs=3) as wp,
            tc.tile_pool(name="pin", bufs=8) as pinp,
            tc.tile_pool(name="pout", bufs=4) as poutp,
            tc.tile_pool(name="tmid", bufs=2) as tmidp,
            tc.tile_pool(name="ps", bufs=8, space="PSUM") as psp,
        ):
            # ---- persistent SBUF tiles ----
            decT_sb = cp.tile([P, KC, TQH], F32, tag="decT_sb")
            decTb_sb = cp.tile([P, KC, TQH], BF16, tag="decTb_sb")
            encTb_sb = cp.tile([P, KC, TS], BF16, tag="encTb_sb")
            wfcqb_sb = cp.tile([P, KC, D], BF16, tag="wfcqb_sb")
            wqb_sb = cp.tile([P, KC, D], BF16, tag="wqb_sb")
            wkb_sb = cp.tile([P, KC, D], BF16, tag="wkb_sb")
            pk_sb = cp.tile([P, PK], F32, tag="pk_sb")
            ident_sb = cp.tile([P, P], F32, tag="ident_sb")
            identb_sb = cp.tile([P, P], BF16, tag="identb_sb")
            srcrow_sb = cp.tile([P, TS], F32, tag="srcrow_sb")
            invcntrow_sb = cp.tile([P, TS], F32, tag="invcntrow_sb")
            Dm_sb = cp.tile([P, SC, TS], BF16, tag="Dm_sb")
            cnt_sb = cp.tile([P, SC], F32, tag="cnt_sb")
            invcnt_sb = cp.tile([P, SC], F32, tag="invcnt_sb")
            cqTb_sb = cp.tile([P, KC, TQH], BF16, tag="cqTb_sb")
            qTb_sb = cp.tile([P, KC, TQH], BF16, tag="qTb_sb")
            kTb_sb = cp.tile([P, KC, TS], BF16, tag="kTb_sb")
            attn_sb = cp.tile([P, MI, TS], BF16, tag="attn_sb")
            attnT_sb = cp.tile([P, SC, TQH], BF16, tag="attnT_sb")
            e_sb = cp.tile([P, MI, TS], F32, tag="e_sb")
            p1c_sb = cp.tile([P, MI, TS], BF16, tag="p1c_sb")
            fix_sb = cp.tile([P, MI, TS], F32, tag="fix_sb")
            sume_sb = cp.tile([P, MI], F32, tag="sume_sb")
            denom_sb = cp.tile([P, MI], F32, tag="denom_sb")
            rden_sb = cp.tile([P, MI], F32, tag="rden_sb")
            w_sb = cp.tile([P, MI], F32, tag="w_sb")
            s1_sb = cp.tile([P, MI], F32, tag="s1_sb")
            s2_sb = cp.tile([P, MI], F32, tag="s2_sb")

            srcf_sb = pk_sb[:, 0:4]
            bfcq_sb = pk_sb[:, 4:8]
            bq_sb = pk_sb[:, 8:12]
            bk_sb = pk_sb[:, 12:16]
            wfcw_sb = pk_sb[:, 16:20]
            bfcw_sb = pk_sb[:, 20:21]

            # ---- loads: everything except the p1 stream rides the scalar
            #      ring (idle until the first out-store ~20us in); the sync
            #      ring is a pure p1 bf16 stream from instruction 0 ----
            nc.scalar.dma_start(out=pk_sb[:], in_=pk[:])
            nc.scalar.dma_start(
                out=wfcqb_sb[:], in_=wfcqb[:].rearrange("(c p) q -> p c q", p=P)
            )
            nc.scalar.dma_start(
                out=decTb_sb[:], in_=decTb[:].rearrange("(c p) q -> p c q", p=P)
            )
            nc.scalar.dma_start(
                out=wqb_sb[:], in_=wqb[:].rearrange("(c p) q -> p c q", p=P)
            )
            nc.scalar.dma_start(
                out=encTb_sb[:], in_=encTb[:].rearrange("(c p) q -> p c q", p=P)
            )
            nc.scalar.dma_start(
                out=wkb_sb[:], in_=wkb[:].rearrange("(c p) q -> p c q", p=P)
            )
            nc.scalar.dma_start(
                out=decT_sb[:], in_=decT[:].rearrange("(c p) q -> p c q", p=P)
            )
            nc.scalar.dma_start(
                out=p1c_sb[:], in_=p1c[:].rearrange("(mi p) s -> p mi s", p=P)
            )
            make_identity(nc, ident_sb[:])
            make_identity(nc, identb_sb[:])

            # ---- selection matrix Dm, counts (only needs srcf) ----
            for c in range(SC):
                pt = psp.tile([P, P], F32, tag="ps")
                nc.tensor.transpose(
                    out=pt[:],
                    in_=srcf_sb[:, c : c + 1].to_broadcast([P, P]),
                    identity=ident_sb[:],
                )
                nc.vector.tensor_copy(srcrow_sb[:, c * P : (c + 1) * P], pt[:])
            for a in range(SC):
                nc.vector.tensor_tensor(
                    out=Dm_sb[:, a, :],
                    in0=srcf_sb[:, a : a + 1].to_broadcast([P, TS]),
                    in1=srcrow_sb[:],
                    op=ALU.is_equal,
                )
                nc.vector.tensor_reduce(
                    cnt_sb[:, a : a + 1], Dm_sb[:, a, :], AX.X, ALU.add
                )
            nc.vector.reciprocal(invcnt_sb[:], cnt_sb[:])
            for c in range(SC):
                pt = psp.tile([P, P], F32, tag="ps")
                nc.tensor.transpose(
                    out=pt[:],
                    in_=invcnt_sb[:, c : c + 1].to_broadcast([P, P]),
                    identity=ident_sb[:],
                )
                nc.vector.tensor_copy(invcntrow_sb[:, c * P : (c + 1) * P], pt[:])

            # ---- cq_T = WfcQ @ dec.T + bfcQ  -> [dout, q] (bf16 out) ----
            for mc in range(KC):
                ps = psp.tile([P, TQH], F32, tag="ps")
                for kc in range(KC):
                    nc.tensor.matmul(
                        out=ps[:],
                        lhsT=wfcqb_sb[:, kc, mc * P : (mc + 1) * P],
                        rhs=decTb_sb[:, kc, :],
                        start=(kc == 0),
                        stop=(kc == KC - 1),
                    )
                nc.scalar.activation(
                    cqTb_sb[:, mc, :], ps[:], AF.Identity,
                    bias=bfcq_sb[:, mc : mc + 1], scale=1.0,
                )

            # ---- Q_T = Wq @ cq_T + bq ----
            for mc in range(KC):
                ps = psp.tile([P, TQH], F32, tag="ps")
                for kc in range(KC):
                    nc.tensor.matmul(
                        out=ps[:],
                        lhsT=wqb_sb[:, kc, mc * P : (mc + 1) * P],
                        rhs=cqTb_sb[:, kc, :],
                        start=(kc == 0),
                        stop=(kc == KC - 1),
                    )
                nc.scalar.activation(
                    qTb_sb[:, mc, :], ps[:], AF.Identity,
                    bias=bq_sb[:, mc : mc + 1], scale=1.0,
                )

            # ---- K_T = Wk @ enc.T + bk ----
            for mc in range(KC):
                ps = psp.tile([P, TS], F32, tag="ps")
                for kc in range(KC):
                    nc.tensor.matmul(
                        out=ps[:],
                        lhsT=wkb_sb[:, kc, mc * P : (mc + 1) * P],
                        rhs=encTb_sb[:, kc, :],
                        start=(kc == 0),
                        stop=(kc == KC - 1),
                    )
                nc.scalar.activation(
                    kTb_sb[:, mc, :], ps[:], AF.Identity,
                    bias=bk_sb[:, mc : mc + 1], scale=1.0,
                )

            # ---- w = sigmoid(dec @ Wfcw.T + bfcw) (fp32 matmul); s1 = 1-w ----
            for mi in range(MI):
                ps = psp.tile([P, 1], F32, tag="ps")
                for kc in range(KC):
                    nc.tensor.matmul(
                        out=ps[:],
                        lhsT=decT_sb[:, kc, mi * P : (mi + 1) * P],
                        rhs=wfcw_sb[:, kc : kc + 1],
                        start=(kc == 0),
                        stop=(kc == KC - 1),
                    )
                nc.scalar.activation(
                    w_sb[:, mi : mi + 1], ps[:], AF.Sigmoid,
                    bias=bfcw_sb[:, 0:1], scale=1.0,
                )
                nc.vector.tensor_scalar(
                    out=s1_sb[:, mi : mi + 1], in0=w_sb[:, mi : mi + 1],
                    scalar1=-1.0, scalar2=1.0, op0=ALU.mult, op1=ALU.add,
                )

            p1_v = p1[:].rearrange("(mi p) v -> p mi v", p=P)
            out_v = out[:].rearrange("(mi p) v -> p mi v", p=P)

            def blend_tile(mi, vt, path):
                vs = slice(vt * VT, (vt + 1) * VT)
                pin = pinp.tile([P, VT], BF16, tag="pin")
                nc.sync.dma_start(out=pin[:], in_=p1_v[:, mi, vs])
                pout = poutp.tile([P, VT], BF16, tag="pout")
                if path == "act":
                    # one scalar-engine op, fp32 internal, single bf16 round
                    nc.scalar.activation(
                        pout[:], pin[:], AF.Identity,
                        bias=s2_sb[:, mi : mi + 1],
                        scale=s1_sb[:, mi : mi + 1],
                    )
                else:
                    # DVE pair with fp32 intermediate: also a single bf16 round
                    t = tmidp.tile([P, VT], F32, tag="tmid")
                    nc.vector.tensor_scalar_mul(
                        t[:], pin[:], s1_sb[:, mi : mi + 1]
                    )
                    nc.vector.tensor_scalar_add(
                        pout[:], t[:], s2_sb[:, mi : mi + 1]
                    )
                nc.scalar.dma_start(out=out_v[:, mi, vs], in_=pout[:])

            def attn_chain(mi):
                # scores + per-head softmax (no max subtraction: |logit| is a
                # ~N(0,1) sample, exp is safe in fp32); accumulate the sum of
                # per-head softmaxes (the 1/NH head-mean folds into the
                # e = exp(a_comb/NH) scale below)
                for h in range(NH):
                    hc, hp = h // 2, h % 2
                    ps = psp.tile([P, TS], F32, tag="ps")
                    nc.tensor.matmul(
                        out=ps[:],
                        lhsT=qTb_sb[hp * DH : (hp + 1) * DH, hc, mi * P : (mi + 1) * P],
                        rhs=kTb_sb[hp * DH : (hp + 1) * DH, hc, :],
                        start=True,
                        stop=True,
                    )
                    ex = wp.tile([P, TS], BF16, tag="ex")
                    se = wp.tile([P, 1], F32, tag="se")
                    nc.scalar.activation(
                        ex[:], ps[:], AF.Exp,
                        bias=0.0, scale=0.125, accum_out=se[:, 0:1],
                    )
                    r8 = wp.tile([P, 1], F32, tag="r8")
                    nc.vector.reciprocal(r8[:], se[:, 0:1])
                    if h == 0:
                        nc.vector.tensor_scalar_mul(attn_sb[:, mi, :], ex[:], r8[:, 0:1])
                    else:
                        nc.vector.scalar_tensor_tensor(
                            out=attn_sb[:, mi, :],
                            in0=ex[:],
                            scalar=r8[:, 0:1],
                            in1=attn_sb[:, mi, :],
                            op0=ALU.mult,
                            op1=ALU.add,
                        )

                # attn_T via PE transpose (for the a_comb contraction)
                for sc in range(SC):
                    pt = psp.tile([P, P], BF16, tag="ps")
                    nc.tensor.transpose(
                        out=pt[:],
                        in_=attn_sb[:, mi, sc * P : (sc + 1) * P],
                        identity=identb_sb[:],
                    )
                    nc.vector.tensor_copy(attnT_sb[:, sc, mi * P : (mi + 1) * P], pt[:])

                # a_comb = attn @ Dm ; e = exp(a_comb/NH) ; denom ; s2
                ps = psp.tile([P, TS], F32, tag="ps")
                for c in range(SC):
                    nc.tensor.matmul(
                        out=ps[:],
                        lhsT=attnT_sb[:, c, mi * P : (mi + 1) * P],
                        rhs=Dm_sb[:, c, :],
                        start=(c == 0),
                        stop=(c == SC - 1),
                    )
                nc.scalar.activation(
                    e_sb[:, mi, :], ps[:], AF.Exp, bias=0.0, scale=1.0 / NH
                )
                g = wp.tile([P, TS], F32, tag="g")
                nc.vector.scalar_tensor_tensor(
                    out=g[:],
                    in0=e_sb[:, mi, :],
                    scalar=-1.0,
                    in1=invcntrow_sb[:],
                    op0=ALU.add,
                    op1=ALU.mult,
                )
                nc.vector.tensor_reduce(sume_sb[:, mi : mi + 1], g[:], AX.X, ALU.add)
                nc.vector.tensor_scalar_add(
                    denom_sb[:, mi : mi + 1], sume_sb[:, mi : mi + 1], float(V)
                )
                nc.vector.reciprocal(rden_sb[:, mi : mi + 1], denom_sb[:, mi : mi + 1])
                nc.vector.tensor_tensor(
                    out=s2_sb[:, mi : mi + 1], in0=w_sb[:, mi : mi + 1],
                    in1=rden_sb[:, mi : mi + 1], op=ALU.mult,
                )

            # mi=0 chain, early mi=0 blend (on DVE so the scalar engine is
            # free for mi=1's softmax exps), mi=1 chain, remaining blend
            attn_chain(0)
            for vt in range(4):
                blend_tile(0, vt, "dve")
            attn_chain(1)
            for vt in range(4, NVT):
                blend_tile(0, vt, "act")
            for vt in range(2):
                blend_tile(1, vt, "dve")
            for vt in range(2, NVT):
                blend_tile(1, vt, "act")

            # ---- fix columns: fix = s1*p1c + s2*e  (per-partition scalars;
            #      DMAs ride the scalar-engine HWDGE queue) ----
            for mi in range(MI):
                t2 = wp.tile([P, TS], F32, tag="fix_t2")
                nc.vector.tensor_scalar_mul(t2[:], e_sb[:, mi, :], s2_sb[:, mi : mi + 1])
                nc.vector.scalar_tensor_tensor(
                    out=fix_sb[:, mi, :],
                    in0=p1c_sb[:, mi, :],
                    scalar=s1_sb[:, mi : mi + 1],
                    op0=ALU.mult,
                    in1=t2[:],
                    op1=ALU.add,
                )
            nc.scalar.dma_start(
                out=fixc[:].rearrange("(mi p) s -> p mi s", p=P), in_=fix_sb[:]
            )

    nc.finalize()
    return nc


def _get_nc():
    global _NC_CACHE
    if _NC_CACHE is None:
        _NC_CACHE = build_nc()
    return _NC_CACHE


def kernel(**inputs) -> np.ndarray:
    dec = np.asarray(inputs["dec_output"], dtype=np.float32)  # [4, 512, 512]
    enc = np.asarray(inputs["enc_output"], dtype=np.float32)  # [4, 512, 512]
    src = np.asarray(inputs["src"]).astype(np.int32)  # [4, 512]
    p1 = np.asarray(inputs["p1"], dtype=np.float32)  # [4, 512, 32000]
    WfcQ = np.asarray(inputs["WfcQ"], dtype=np.float32)
    bfcQ = np.asarray(inputs["bfcQ"], dtype=np.float32)
    Wq = np.asarray(inputs["Wq"], dtype=np.float32)
    bq = np.asarray(inputs["bq"], dtype=np.float32)
    Wk = np.asarray(inputs["Wk"], dtype=np.float32)
    bk = np.asarray(inputs["bk"], dtype=np.float32)
    Wfcw = np.asarray(inputs["Wfcw"], dtype=np.float32)
    bfcw = np.asarray(inputs["bfcw"], dtype=np.float32)

    B, TQ, _ = dec.shape
    n_cores = 8

    import ml_dtypes

    bf16 = ml_dtypes.bfloat16
    wfcqb = np.ascontiguousarray(WfcQ.T.astype(bf16))
    wqb = np.ascontiguousarray(Wq.T.astype(bf16))
    wkb = np.ascontiguousarray(Wk.T.astype(bf16))

    in_maps = []
    for core in range(n_cores):
        b, qh = core // 2, core % 2
        qs = slice(qh * TQH, (qh + 1) * TQH)
        p1_slab = p1[b, qs, :]
        # packed per-partition constants: [p, c] = x[c*128 + p]
        pk = np.zeros((P, PK), np.float32)
        pk[:, 0:4] = src[b].reshape(SC, P).T
        pk[:, 4:8] = bfcQ.reshape(KC, P).T
        pk[:, 8:12] = bq.reshape(KC, P).T
        pk[:, 12:16] = bk.reshape(KC, P).T
        pk[:, 16:20] = Wfcw[0].reshape(KC, P).T
        pk[:, 20] = bfcw[0]
        in_maps.append(
            {
                "decT": np.ascontiguousarray(dec[b].T[:, qs]),
                "decTb": np.ascontiguousarray(dec[b].T[:, qs].astype(bf16)),
                "encTb": np.ascontiguousarray(enc[b].T.astype(bf16)),
                "wfcqb": wfcqb,
                "wqb": wqb,
                "wkb": wkb,
                "pk": pk,
                "p1": np.ascontiguousarray(p1_slab.astype(bf16)),
                "p1c": np.ascontiguousarray(p1_slab[:, src[b]].astype(bf16)),
            }
        )

    nc = _get_nc()
    res = run_bass_kernel_spmd(nc, in_maps, core_ids=list(range(n_cores)))
    global _LAST_RESULTS
    _LAST_RESULTS = res

    out = np.empty((B, TQ, V), dtype=np.float32)
    for core in range(n_cores):
        b, qh = core // 2, core % 2
        qs = slice(qh * TQH, (qh + 1) * TQH)
        out[b, qs, :] = res.results[core]["out"].astype(np.float32)
        # place the corrected source-token columns (duplicates carry
        # identical values, so overwrite order does not matter)
        out[b, qs, :][:, src[b]] = res.results[core]["fixc"]
    return out


# revision 24
# speedup vs baseline: 1.0311x; 1.0311x over previous
"""CopyDecoder Trainium2 kernel (nn_CopyDecoder_5274219840242).

Sharding: 8 cores = 4 batches x 2 query-halves (data parallel, no collectives).

Per core (b, q-slab of 256 rows):
  - attention: Q/K projections (fcQ folded into Wq on the host:
    Q = dec @ (Wq@WfcQ).T + (Wq@bfcQ + bq); computed transposed so the
    contraction dim lands on partitions; bf16 operands, fp32 accumulate),
    per-head softmax (logits bounded, so no max-subtraction), head mean.
  - duplicate-combining selection matrix Dm[s,s'] = [src_s == src_s'] built by
    compare-vs-transpose; a_comb = attn @ Dm gives each source position the
    full scatter-sum of its token; e = exp(a_comb/NH).
  - denom[q] = V + sum_s (e[q,s]-1)/cnt[s]  (softmax denominator over vocab,
    exploiting exp(0)=1 for vocab entries no source token maps to).
  - streaming blend over p1 in BF16 both directions (tolerance is 2e-2):
    out = (1-w)*p1 + w/denom.  Halves HBM traffic vs fp32 streaming, which
    is the roofline here (~99% DMA active in the fp32 baseline trace).
    The fused DVE tensor_scalar double-rounds (bf16 intermediate), which
    costs ~1.1e-2 rel err; instead each tile takes a single-rounding path:
    either one scalar-engine activation (out = Identity(p1*s1 + s2), fp32
    internal, per-partition scale/bias APs) or a DVE pair (mul to fp32
    intermediate, add to bf16).  Tiles are split between the two engines
    so neither becomes the bottleneck.
  - fix values for the <=512 source-token columns:
    fix[q,s] = (1-w)*p1[q,src_s] + (w/denom)*e[q,s]
    (p1 columns are host-gathered fp32 into an extra input; the host writes
    the fix columns into the final output during unshard).

Queue split (two HWDGE rings share 16 DMA engines, ~23.5GB/s each busy):
  - sync ring: Q-side weights (wqcb, decTb) first, then the pure p1 bf16
    load stream.
  - scalar ring: packed constants, K-side weights (wkb, encTb), decT,
    p1c, then all out-stores + fixc.
Weights ride ahead of the p1 stream on both rings so the attention chain
(which gates the first store via s2) starts ~5us in, not ~25us.  The
chain runs per q-partition-tile (mi) so the first blend stores start
while the second tile's softmax is still in flight.
"""

import sys

sys.path.insert(0, "/opt/trn_rl_repo")

import numpy as np

import concourse.bacc as bacc
import concourse.bass as bass
import concourse.mybir as mybir
import concourse.tile as tile
from concourse.bass_utils import run_bass_kernel_spmd
from concourse.masks import make_identity

P = 128
D = 512
TS = 512
TQH = 256  # q rows per core
V = 32000
NH = 8
DH = 64
KC = D // P  # 4 contraction chunks
MI = TQH // P  # 2 q partition tiles
SC = TS // P  # 4 source-position chunks
VT = 4000  # vocab columns per blend tile (8000B bf16 per partition row)
NVT = V // VT  # 8 vocab tiles per q partition tile

F32 = mybir.dt.float32
BF16 = mybir.dt.bfloat16
I32 = mybir.dt.int32
AF = mybir.ActivationFunctionType
ALU = mybir.AluOpType
AX = mybir.AxisListType

# packed per-partition constants layout (f32 columns):
#   [0:4) srcf   [4:8) bqc   [8:12) bk   [12:16) wfcw   [16] bfcw
PK = 17

_NC_CACHE = None
_LAST_RESULTS = None


def build_nc():
    nc = bacc.Bacc("TRN2", target_bir_lowering=False, debug=False)

    decT = nc.dram_tensor("decT", [D, TQH], F32, kind="ExternalInput")
    decTb = nc.dram_tensor("decTb", [D, TQH], BF16, kind="ExternalInput")
    encTb = nc.dram_tensor("encTb", [D, TS], BF16, kind="ExternalInput")
    wqcb = nc.dram_tensor("wqcb", [D, D], BF16, kind="ExternalInput")
    wkb = nc.dram_tensor("wkb", [D, D], BF16, kind="ExternalInput")
    pk = nc.dram_tensor("pk", [P, PK], F32, kind="ExternalInput")
    p1 = nc.dram_tensor("p1", [TQH, V], BF16, kind="ExternalInput")
    p1c = nc.dram_tensor("p1c", [TQH, TS], BF16, kind="ExternalInput")
    out = nc.dram_tensor("out", [TQH, V], BF16, kind="ExternalOutput")
    fixc = nc.dram_tensor("fixc", [TQH, TS], F32, kind="ExternalOutput")

    with tile.TileContext(nc) as tc:
        with (
            tc.tile_pool(name="const", bufs=1) as cp,
            tc.tile_pool(name="work", bufs=3) as wp,
            tc.tile_pool(name="pin", bufs=8) as pinp,
            tc.tile_pool(name="pout", bufs=4) as poutp,
            tc.tile_pool(name="tmid", bufs=2) as tmidp,
            tc.tile_pool(name="ps", bufs=8, space="PSUM") as psp,
        ):
            # ---- persistent SBUF tiles ----
            decT_sb = cp.tile([P, KC, TQH], F32, tag="decT_sb")
            decTb_sb = cp.tile([P, KC, TQH], BF16, tag="decTb_sb")
            encTb_sb = cp.tile([P, KC, TS], BF16, tag="encTb_sb")
            wqcb_sb = cp.tile([P, KC, D], BF16, tag="wqcb_sb")
            wkb_sb = cp.tile([P, KC, D], BF16, tag="wkb_sb")
            pk_sb = cp.tile([P, PK], F32, tag="pk_sb")
            ident_sb = cp.tile([P, P], F32, tag="ident_sb")
            identb_sb = cp.tile([P, P], BF16, tag="identb_sb")
            srcrow_sb = cp.tile([P, TS], F32, tag="srcrow_sb")
            invcntrow_sb = cp.tile([P, TS], F32, tag="invcntrow_sb")
            Dm_sb = cp.tile([P, SC, TS], BF16, tag="Dm_sb")
            cnt_sb = cp.tile([P, SC], F32, tag="cnt_sb")
            invcnt_sb = cp.tile([P, SC], F32, tag="invcnt_sb")
            qTb_sb = cp.tile([P, KC, TQH], BF16, tag="qTb_sb")
            kTb_sb = cp.tile([P, KC, TS], BF16, tag="kTb_sb")
            attn_sb = cp.tile([P, MI, TS], BF16, tag="attn_sb")
            attnT_sb = cp.tile([P, SC, TQH], BF16, tag="attnT_sb")
            e_sb = cp.tile([P, MI, TS], F32, tag="e_sb")
            p1c_sb = cp.tile([P, MI, TS], BF16, tag="p1c_sb")
            fix_sb = cp.tile([P, MI, TS], F32, tag="fix_sb")
            sume_sb = cp.tile([P, MI], F32, tag="sume_sb")
            denom_sb = cp.tile([P, MI], F32, tag="denom_sb")
            rden_sb = cp.tile([P, MI], F32, tag="rden_sb")
            w_sb = cp.tile([P, MI], F32, tag="w_sb")
            s1_sb = cp.tile([P, MI], F32, tag="s1_sb")
            s2_sb = cp.tile([P, MI], F32, tag="s2_sb")

            srcf_sb = pk_sb[:, 0:4]
            bqc_sb = pk_sb[:, 4:8]
            bk_sb = pk_sb[:, 8:12]
            wfcw_sb = pk_sb[:, 12:16]
            bfcw_sb = pk_sb[:, 16:17]

            # ---- loads: Q-side operands lead the sync ring (ahead of the
            #      p1 stream); K-side operands + the rest lead the scalar
            #      ring (ahead of the out-stores) ----
            nc.sync.dma_start(
                out=wqcb_sb[:], in_=wqcb[:].rearrange("(c p) q -> p c q", p=P)
            )
            nc.sync.dma_start(
                out=decTb_sb[:], in_=decTb[:].rearrange("(c p) q -> p c q", p=P)
            )
            nc.scalar.dma_start(out=pk_sb[:], in_=pk[:])
            nc.scalar.dma_start(
                out=wkb_sb[:], in_=wkb[:].rearrange("(c p) q -> p c q", p=P)
            )
            nc.scalar.dma_start(
                out=encTb_sb[:], in_=encTb[:].rearrange("(c p) q -> p c q", p=P)
            )
            nc.scalar.dma_start(
                out=decT_sb[:], in_=decT[:].rearrange("(c p) q -> p c q", p=P)
            )
            nc.scalar.dma_start(
                out=p1c_sb[:], in_=p1c[:].rearrange("(mi p) s -> p mi s", p=P)
            )
            make_identity(nc, ident_sb[:])
            make_identity(nc, identb_sb[:])

            # ---- selection matrix Dm, counts (only needs srcf) ----
            for c in range(SC):
                pt = psp.tile([P, P], F32, tag="ps")
                nc.tensor.transpose(
                    out=pt[:],
                    in_=srcf_sb[:, c : c + 1].to_broadcast([P, P]),
                    identity=ident_sb[:],
                )
                nc.vector.tensor_copy(srcrow_sb[:, c * P : (c + 1) * P], pt[:])
            for a in range(SC):
                nc.vector.tensor_tensor(
                    out=Dm_sb[:, a, :],
                    in0=srcf_sb[:, a : a + 1].to_broadcast([P, TS]),
                    in1=srcrow_sb[:],
                    op=ALU.is_equal,
                )
                nc.vector.tensor_reduce(
                    cnt_sb[:, a : a + 1], Dm_sb[:, a, :], AX.X, ALU.add
                )
            nc.vector.reciprocal(invcnt_sb[:], cnt_sb[:])
            for c in range(SC):
                pt = psp.tile([P, P], F32, tag="ps")
                nc.tensor.transpose(
                    out=pt[:],
                    in_=invcnt_sb[:, c : c + 1].to_broadcast([P, P]),
                    identity=ident_sb[:],
                )
                nc.vector.tensor_copy(invcntrow_sb[:, c * P : (c + 1) * P], pt[:])

            # ---- Q_T = Wqc @ dec.T + bqc  (fcQ folded into Wq host-side) ----
            for mc in range(KC):
                ps = psp.tile([P, TQH], F32, tag="ps")
                for kc in range(KC):
                    nc.tensor.matmul(
                        out=ps[:],
                        lhsT=wqcb_sb[:, kc, mc * P : (mc + 1) * P],
                        rhs=decTb_sb[:, kc, :],
                        start=(kc == 0),
                        stop=(kc == KC - 1),
                    )
                nc.scalar.activation(
                    qTb_sb[:, mc, :], ps[:], AF.Identity,
                    bias=bqc_sb[:, mc : mc + 1], scale=1.0,
                )

            # ---- K_T = Wk @ enc.T + bk ----
            for mc in range(KC):
                ps = psp.tile([P, TS], F32, tag="ps")
                for kc in range(KC):
                    nc.tensor.matmul(
                        out=ps[:],
                        lhsT=wkb_sb[:, kc, mc * P : (mc + 1) * P],
                        rhs=encTb_sb[:, kc, :],
                        start=(kc == 0),
                        stop=(kc == KC - 1),
                    )
                nc.scalar.activation(
                    kTb_sb[:, mc, :], ps[:], AF.Identity,
                    bias=bk_sb[:, mc : mc + 1], scale=1.0,
                )

            # ---- w = sigmoid(dec @ Wfcw.T + bfcw) (fp32 matmul); s1 = 1-w ----
            for mi in range(MI):
                ps = psp.tile([P, 1], F32, tag="ps")
                for kc in range(KC):
                    nc.tensor.matmul(
                        out=ps[:],
                        lhsT=decT_sb[:, kc, mi * P : (mi + 1) * P],
                        rhs=wfcw_sb[:, kc : kc + 1],
                        start=(kc == 0),
                        stop=(kc == KC - 1),
                    )
                nc.scalar.activation(
                    w_sb[:, mi : mi + 1], ps[:], AF.Sigmoid,
                    bias=bfcw_sb[:, 0:1], scale=1.0,
                )
                nc.vector.tensor_scalar(
                    out=s1_sb[:, mi : mi + 1], in0=w_sb[:, mi : mi + 1],
                    scalar1=-1.0, scalar2=1.0, op0=ALU.mult, op1=ALU.add,
                )

            p1_v = p1[:].rearrange("(mi p) v -> p mi v", p=P)
            out_v = out[:].rearrange("(mi p) v -> p mi v", p=P)

            def blend_tile(mi, vt, path):
                vs = slice(vt * VT, (vt + 1) * VT)
                pin = pinp.tile([P, VT], BF16, tag="pin")
                nc.sync.dma_start(out=pin[:], in_=p1_v[:, mi, vs])
                pout = poutp.tile([P, VT], BF16, tag="pout")
                if path == "act":
                    # one scalar-engine op, fp32 internal, single bf16 round
                    nc.scalar.activation(
                        pout[:], pin[:], AF.Identity,
                        bias=s2_sb[:, mi : mi + 1],
                        scale=s1_sb[:, mi : mi + 1],
                    )
                else:
                    # DVE pair with fp32 intermediate: also a single bf16 round
                    t = tmidp.tile([P, VT], F32, tag="tmid")
                    nc.vector.tensor_scalar_mul(
                        t[:], pin[:], s1_sb[:, mi : mi + 1]
                    )
                    nc.vector.tensor_scalar_add(
                        pout[:], t[:], s2_sb[:, mi : mi + 1]
                    )
                nc.scalar.dma_start(out=out_v[:, mi, vs], in_=pout[:])

            def attn_chain(mi):
                # scores + per-head softmax (no max subtraction: |logit| is a
                # ~N(0,1) sample, exp is safe in fp32); accumulate the sum of
                # per-head softmaxes (the 1/NH head-mean folds into the
                # e = exp(a_comb/NH) scale below)
                for h in range(NH):
                    hc, hp = h // 2, h % 2
                    ps = psp.tile([P, TS], F32, tag="ps")
                    nc.tensor.matmul(
                        out=ps[:],
                        lhsT=qTb_sb[hp * DH : (hp + 1) * DH, hc, mi * P : (mi + 1) * P],
                        rhs=kTb_sb[hp * DH : (hp + 1) * DH, hc, :],
                        start=True,
                        stop=True,
                    )
                    ex = wp.tile([P, TS], BF16, tag="ex")
                    se = wp.tile([P, 1], F32, tag="se")
                    nc.scalar.activation(
                        ex[:], ps[:], AF.Exp,
                        bias=0.0, scale=0.125, accum_out=se[:, 0:1],
                    )
                    r8 = wp.tile([P, 1], F32, tag="r8")
                    nc.vector.reciprocal(r8[:], se[:, 0:1])
                    if h == 0:
                        nc.vector.tensor_scalar_mul(attn_sb[:, mi, :], ex[:], r8[:, 0:1])
                    else:
                        nc.vector.scalar_tensor_tensor(
                            out=attn_sb[:, mi, :],
                            in0=ex[:],
                            scalar=r8[:, 0:1],
                            in1=attn_sb[:, mi, :],
                            op0=ALU.mult,
                            op1=ALU.add,
                        )

                # attn_T via PE transpose (for the a_comb contraction)
                for sc in range(SC):
                    pt = psp.tile([P, P], BF16, tag="ps")
                    nc.tensor.transpose(
                        out=pt[:],
                        in_=attn_sb[:, mi, sc * P : (sc + 1) * P],
                        identity=identb_sb[:],
                    )
                    nc.vector.tensor_copy(attnT_sb[:, sc, mi * P : (mi + 1) * P], pt[:])

                # a_comb = attn @ Dm ; e = exp(a_comb/NH) ; denom ; s2
                ps = psp.tile([P, TS], F32, tag="ps")
                for c in range(SC):
                    nc.tensor.matmul(
                        out=ps[:],
                        lhsT=attnT_sb[:, c, mi * P : (mi + 1) * P],
                        rhs=Dm_sb[:, c, :],
                        start=(c == 0),
                        stop=(c == SC - 1),
                    )
                nc.scalar.activation(
                    e_sb[:, mi, :], ps[:], AF.Exp, bias=0.0, scale=1.0 / NH
                )
                g = wp.tile([P, TS], F32, tag="g")
                nc.vector.scalar_tensor_tensor(
                    out=g[:],
                    in0=e_sb[:, mi, :],
                    scalar=-1.0,
                    in1=invcntrow_sb[:],
                    op0=ALU.add,
                    op1=ALU.mult,
                )
                nc.vector.tensor_reduce(sume_sb[:, mi : mi + 1], g[:], AX.X, ALU.add)
                nc.vector.tensor_scalar_add(
                    denom_sb[:, mi : mi + 1], sume_sb[:, mi : mi + 1], float(V)
                )
                nc.vector.reciprocal(rden_sb[:, mi : mi + 1], denom_sb[:, mi : mi + 1])
                nc.vector.tensor_tensor(
                    out=s2_sb[:, mi : mi + 1], in0=w_sb[:, mi : mi + 1],
                    in1=rden_sb[:, mi : mi + 1], op=ALU.mult,
                )

            # mi=0 chain, two early mi=0 blends (one per engine), mi=1 chain,
            # then the rest alternating act/dve so both engines blend
            # concurrently and neither starves the store stream
            attn_chain(0)
            blend_tile(0, 0, "act")
            blend_tile(0, 1, "dve")
            attn_chain(1)
            for vt in range(2, NVT):
                blend_tile(0, vt, "act" if vt % 2 == 0 else "dve")
            for vt in range(NVT):
                blend_tile(1, vt, "act" if vt % 2 == 0 else "dve")

            # ---- fix columns: fix = s1*p1c + s2*e  (per-partition scalars;
            #      DMAs ride the scalar-engine HWDGE queue) ----
            for mi in range(MI):
                t2 = wp.tile([P, TS], F32, tag="fix_t2")
                nc.vector.tensor_scalar_mul(t2[:], e_sb[:, mi, :], s2_sb[:, mi : mi + 1])
                nc.vector.scalar_tensor_tensor(
                    out=fix_sb[:, mi, :],
                    in0=p1c_sb[:, mi, :],
                    scalar=s1_sb[:, mi : mi + 1],
                    op0=ALU.mult,
                    in1=t2[:],
                    op1=ALU.add,
                )
            nc.scalar.dma_start(
                out=fixc[:].rearrange("(mi p) s -> p mi s", p=P), in_=fix_sb[:]
            )

    nc.finalize()
    return nc


def _get_nc():
    global _NC_CACHE
    if _NC_CACHE is None:
        _NC_CACHE = build_nc()
    return _NC_CACHE


def kernel(**inputs) -> np.ndarray:
    dec = np.asarray(inputs["dec_output"], dtype=np.float32)  # [4, 512, 512]
    enc = np.asarray(inputs["enc_output"], dtype=np.float32)  # [4, 512, 512]
    src = np.asarray(inputs["src"]).astype(np.int32)  # [4, 512]
    p1 = np.asarray(inputs["p1"], dtype=np.float32)  # [4, 512, 32000]
    WfcQ = np.asarray(inputs["WfcQ"], dtype=np.float32)
    bfcQ = np.asarray(inputs["bfcQ"], dtype=np.float32)
    Wq = np.asarray(inputs["Wq"], dtype=np.float32)
    bq = np.asarray(inputs["bq"], dtype=np.float32)
    Wk = np.asarray(inputs["Wk"], dtype=np.float32)
    bk = np.asarray(inputs["bk"], dtype=np.float32)
    Wfcw = np.asarray(inputs["Wfcw"], dtype=np.float32)
    bfcw = np.asarray(inputs["bfcw"], dtype=np.float32)

    B, TQ, _ = dec.shape
    n_cores = 8

    import ml_dtypes

    bf16 = ml_dtypes.bfloat16
    # fold fcQ into the query projection (cq feeds nothing else)
    Wqc = Wq @ WfcQ
    bqc = Wq @ bfcQ + bq
    wqcb = np.ascontiguousarray(Wqc.T.astype(bf16))
    wkb = np.ascontiguousarray(Wk.T.astype(bf16))

    in_maps = []
    for core in range(n_cores):
        b, qh = core // 2, core % 2
        qs = slice(qh * TQH, (qh + 1) * TQH)
        p1_slab = p1[b, qs, :]
        # packed per-partition constants: [p, c] = x[c*128 + p]
        pk = np.zeros((P, PK), np.float32)
        pk[:, 0:4] = src[b].reshape(SC, P).T
        pk[:, 4:8] = bqc.reshape(KC, P).T
        pk[:, 8:12] = bk.reshape(KC, P).T
        pk[:, 12:16] = Wfcw[0].reshape(KC, P).T
        pk[:, 16] = bfcw[0]
        in_maps.append(
            {
                "decT": np.ascontiguousarray(dec[b].T[:, qs]),
                "decTb": np.ascontiguousarray(dec[b].T[:, qs].astype(bf16)),
                "encTb": np.ascontiguousarray(enc[b].T.astype(bf16)),
                "wqcb": wqcb,
                "wkb": wkb,
                "pk": pk,
                "p1": np.ascontiguousarray(p1_slab.astype(bf16)),
                "p1c": np.ascontiguousarray(p1_slab[:, src[b]].astype(bf16)),
            }
        )

    nc = _get_nc()
    res = run_bass_kernel_spmd(nc, in_maps, core_ids=list(range(n_cores)))
    global _LAST_RESULTS
    _LAST_RESULTS = res

    out = np.empty((B, TQ, V), dtype=np.float32)
    for core in range(n_cores):
        b, qh = core // 2, core % 2
        qs = slice(qh * TQH, (qh + 1) * TQH)
        out[b, qs, :] = res.results[core]["out"].astype(np.float32)
        # place the corrected source-token columns (duplicates carry
        # identical values, so overwrite order does not matter)
        out[b, qs, :][:, src[b]] = res.results[core]["fixc"]
    return out


# revision 30
# speedup vs baseline: 1.1243x; 1.0904x over previous
"""CopyDecoder Trainium2 kernel (nn_CopyDecoder_5274219840242).

Sharding: 8 cores = 4 batches x 2 query-halves (data parallel, no collectives).

Per core (b, q-slab of 256 rows):
  - attention: Q/K projections (fcQ folded into Wq on the host:
    Q = dec @ (Wq@WfcQ).T + (Wq@bfcQ + bq); computed transposed so the
    contraction dim lands on partitions; bf16 operands, fp32 accumulate),
    per-head softmax (logits bounded, so no max-subtraction), head mean.
  - duplicate-combining selection matrix Dm[s,s'] = [src_s == src_s'] built by
    compare-vs-transpose; a_comb = attn @ Dm gives each source position the
    full scatter-sum of its token; e = exp(a_comb/NH).
  - denom[q] = V + sum_s (e[q,s]-1)/cnt[s]  (softmax denominator over vocab,
    exploiting exp(0)=1 for vocab entries no source token maps to).
  - streaming blend over p1 in BF16 both directions (tolerance is 2e-2):
    out = (1-w)*p1 + w/denom.  Halves HBM traffic vs fp32 streaming, which
    is the roofline here (~99% DMA active in the fp32 baseline trace).
    The fused DVE tensor_scalar double-rounds (bf16 intermediate), which
    costs ~1.1e-2 rel err; instead each tile takes a single-rounding path:
    either one scalar-engine activation (out = Identity(p1*s1 + s2), fp32
    internal, per-partition scale/bias APs) or a DVE pair (mul to fp32
    intermediate, add to bf16).  Tiles are split between the two engines
    so neither becomes the bottleneck.
  - fix values for the <=512 source-token columns:
    fix[q,s] = (1-w)*p1[q,src_s] + (w/denom)*e[q,s]
    (p1 columns are host-gathered fp32 into an extra input; the host writes
    the fix columns into the final output during unshard).

Queue split (two HWDGE rings share 16 DMA engines, ~23.5GB/s each busy):
  - sync ring: Q-side weights (wqcb, decTb) first, then the pure p1 bf16
    load stream.
  - scalar ring: packed constants, K-side weights (wkb, encTb), decT,
    p1c, then all out-stores + fixc.
Weights ride ahead of the p1 stream on both rings so the attention chain
(which gates the first store via s2) starts ~5us in, not ~25us.  The
chain runs per q-partition-tile (mi) so the first blend stores start
while the second tile's softmax is still in flight.
"""

import sys

sys.path.insert(0, "/opt/trn_rl_repo")

import numpy as np

import concourse.bacc as bacc
import concourse.bass as bass
import concourse.mybir as mybir
import concourse.tile as tile
from concourse.bass_utils import run_bass_kernel_spmd
from concourse.masks import make_identity

P = 128
D = 512
TS = 512
TQH = 256  # q rows per core
V = 32000
NH = 8
DH = 64
KC = D // P  # 4 contraction chunks
MI = TQH // P  # 2 q partition tiles
SC = TS // P  # 4 source-position chunks
VT = 4000  # vocab columns per blend tile (8000B bf16 per partition row)
NVT = V // VT  # 8 vocab tiles per q partition tile

F32 = mybir.dt.float32
BF16 = mybir.dt.bfloat16
I32 = mybir.dt.int32
AF = mybir.ActivationFunctionType
ALU = mybir.AluOpType
AX = mybir.AxisListType

# packed per-partition constants layout (f32 columns):
#   [0:4) srcf   [4:8) bqc   [8:12) bk   [12:16) wfcw   [16] bfcw
PK = 17

_NC_CACHE = None
_LAST_RESULTS = None


def build_nc():
    nc = bacc.Bacc("TRN2", target_bir_lowering=False, debug=False)

    decT = nc.dram_tensor("decT", [D, TQH], F32, kind="ExternalInput")
    decTb = nc.dram_tensor("decTb", [D, TQH], BF16, kind="ExternalInput")
    encTb = nc.dram_tensor("encTb", [D, TS], BF16, kind="ExternalInput")
    wqcb = nc.dram_tensor("wqcb", [D, D], BF16, kind="ExternalInput")
    wkb = nc.dram_tensor("wkb", [D, D], BF16, kind="ExternalInput")
    pk = nc.dram_tensor("pk", [P, PK], F32, kind="ExternalInput")
    p1 = nc.dram_tensor("p1", [TQH, V], BF16, kind="ExternalInput")
    p1c = nc.dram_tensor("p1c", [TQH, TS], BF16, kind="ExternalInput")
    out = nc.dram_tensor("out", [TQH, V], BF16, kind="ExternalOutput")
    fixc = nc.dram_tensor("fixc", [TQH, TS], F32, kind="ExternalOutput")

    with tile.TileContext(nc) as tc:
        with (
            tc.tile_pool(name="const", bufs=1) as cp,
            tc.tile_pool(name="work", bufs=6) as wp,
            tc.tile_pool(name="pin", bufs=8) as pinp,
            tc.tile_pool(name="pout", bufs=4) as poutp,
            tc.tile_pool(name="tmid", bufs=2) as tmidp,
            tc.tile_pool(name="ps", bufs=8, space="PSUM") as psp,
        ):
            # ---- persistent SBUF tiles ----
            decT_sb = cp.tile([P, KC, TQH], F32, tag="decT_sb")
            decTb_sb = cp.tile([P, KC, TQH], BF16, tag="decTb_sb")
            encTb_sb = cp.tile([P, KC, TS], BF16, tag="encTb_sb")
            wqcb_sb = cp.tile([P, KC, D], BF16, tag="wqcb_sb")
            wkb_sb = cp.tile([P, KC, D], BF16, tag="wkb_sb")
            pk_sb = cp.tile([P, PK], F32, tag="pk_sb")
            ident_sb = cp.tile([P, P], F32, tag="ident_sb")
            identb_sb = cp.tile([P, P], BF16, tag="identb_sb")
            srcrow_sb = cp.tile([P, TS], F32, tag="srcrow_sb")
            invcntrow_sb = cp.tile([P, TS], F32, tag="invcntrow_sb")
            Dm_sb = cp.tile([P, SC, TS], BF16, tag="Dm_sb")
            cnt_sb = cp.tile([P, SC], F32, tag="cnt_sb")
            invcnt_sb = cp.tile([P, SC], F32, tag="invcnt_sb")
            qTb_sb = cp.tile([P, KC, TQH], BF16, tag="qTb_sb")
            kTb_sb = cp.tile([P, KC, TS], BF16, tag="kTb_sb")
            attn_sb = cp.tile([P, MI, TS], BF16, tag="attn_sb")
            attnT_sb = cp.tile([P, SC, TQH], BF16, tag="attnT_sb")
            e_sb = cp.tile([P, MI, TS], F32, tag="e_sb")
            p1c_sb = cp.tile([P, MI, TS], BF16, tag="p1c_sb")
            fix_sb = cp.tile([P, MI, TS], F32, tag="fix_sb")
            sume_sb = cp.tile([P, MI], F32, tag="sume_sb")
            denom_sb = cp.tile([P, MI], F32, tag="denom_sb")
            rden_sb = cp.tile([P, MI], F32, tag="rden_sb")
            w_sb = cp.tile([P, MI], F32, tag="w_sb")
            ez_sb = cp.tile([P, MI], F32, tag="ez_sb")
            t1_sb = cp.tile([P, MI], F32, tag="t1_sb")
            s1_sb = cp.tile([P, MI], F32, tag="s1_sb")
            s2_sb = cp.tile([P, MI], F32, tag="s2_sb")

            srcf_sb = pk_sb[:, 0:4]
            bqc_sb = pk_sb[:, 4:8]
            bk_sb = pk_sb[:, 8:12]
            wfcw_sb = pk_sb[:, 12:16]
            nbfcw_sb = pk_sb[:, 16:17]  # NEGATED gate bias (exp-form gate)

            # ---- loads: Q-side operands lead the sync ring (ahead of the
            #      p1 stream); K-side operands + the rest lead the scalar
            #      ring (ahead of the out-stores).  First column-block of
            #      each weight goes separately so mc=0 matmuls start early.
            wqc_v = wqcb[:].rearrange("(c p) q -> p c q", p=P)
            wk_v = wkb[:].rearrange("(c p) q -> p c q", p=P)
            nc.sync.dma_start(
                out=decTb_sb[:], in_=decTb[:].rearrange("(c p) q -> p c q", p=P)
            )
            nc.sync.dma_start(out=wqcb_sb[:, :, 0:P], in_=wqc_v[:, :, 0:P])
            nc.sync.dma_start(out=wqcb_sb[:, :, P:D], in_=wqc_v[:, :, P:D])
            nc.sync.dma_start(
                out=decT_sb[:], in_=decT[:].rearrange("(c p) q -> p c q", p=P)
            )
            nc.scalar.dma_start(out=pk_sb[:], in_=pk[:])
            nc.scalar.dma_start(
                out=encTb_sb[:], in_=encTb[:].rearrange("(c p) q -> p c q", p=P)
            )
            nc.scalar.dma_start(out=wkb_sb[:, :, 0:P], in_=wk_v[:, :, 0:P])
            nc.scalar.dma_start(out=wkb_sb[:, :, P:D], in_=wk_v[:, :, P:D])
            nc.scalar.dma_start(
                out=p1c_sb[:], in_=p1c[:].rearrange("(mi p) s -> p mi s", p=P)
            )
            make_identity(nc, ident_sb[:])
            make_identity(nc, identb_sb[:])

            # pull the EXP activation table in off the critical path (the
            # scalar engine runs exclusively exps until the blend phase)
            junk = wp.tile([P, 1], F32, tag="junk")
            nc.scalar.activation(junk[:], pk_sb[:, 0:1], AF.Exp, bias=0.0, scale=1.0)

            # ---- selection matrix Dm, counts (only needs srcf) ----
            for c in range(SC):
                pt = psp.tile([P, P], F32, tag="ps")
                nc.tensor.transpose(
                    out=pt[:],
                    in_=srcf_sb[:, c : c + 1].to_broadcast([P, P]),
                    identity=ident_sb[:],
                )
                nc.vector.tensor_copy(srcrow_sb[:, c * P : (c + 1) * P], pt[:])
            for a in range(SC):
                nc.vector.tensor_tensor(
                    out=Dm_sb[:, a, :],
                    in0=srcf_sb[:, a : a + 1].to_broadcast([P, TS]),
                    in1=srcrow_sb[:],
                    op=ALU.is_equal,
                )
                nc.vector.tensor_reduce(
                    cnt_sb[:, a : a + 1], Dm_sb[:, a, :], AX.X, ALU.add
                )
            nc.vector.reciprocal(invcnt_sb[:], cnt_sb[:])
            for c in range(SC):
                pt = psp.tile([P, P], F32, tag="ps")
                nc.tensor.transpose(
                    out=pt[:],
                    in_=invcnt_sb[:, c : c + 1].to_broadcast([P, P]),
                    identity=ident_sb[:],
                )
                nc.vector.tensor_copy(invcntrow_sb[:, c * P : (c + 1) * P], pt[:])

            # ---- per-chunk Q/K projections (bias-add on DVE, so the scalar
            #      engine stays on the exp table) interleaved with the scores
            #      + per-head softmax for the two heads living in that chunk:
            #      softmax pipelines with the projections and BOTH mi chains
            #      finish together ----
            for mc in range(KC):
                psq = psp.tile([P, TQH], F32, tag="ps")
                for kc in range(KC):
                    nc.tensor.matmul(
                        out=psq[:],
                        lhsT=wqcb_sb[:, kc, mc * P : (mc + 1) * P],
                        rhs=decTb_sb[:, kc, :],
                        start=(kc == 0),
                        stop=(kc == KC - 1),
                    )
                nc.vector.tensor_scalar_add(
                    qTb_sb[:, mc, :], psq[:], bqc_sb[:, mc : mc + 1]
                )
                psk = psp.tile([P, TS], F32, tag="ps")
                for kc in range(KC):
                    nc.tensor.matmul(
                        out=psk[:],
                        lhsT=wkb_sb[:, kc, mc * P : (mc + 1) * P],
                        rhs=encTb_sb[:, kc, :],
                        start=(kc == 0),
                        stop=(kc == KC - 1),
                    )
                nc.vector.tensor_scalar_add(
                    kTb_sb[:, mc, :], psk[:], bk_sb[:, mc : mc + 1]
                )
                # heads 2*mc and 2*mc+1 need only chunk mc of Q_T/K_T.
                # logits are ~N(0,1) so exp without max-subtraction is safe;
                # accumulate the sum of per-head softmaxes (the 1/NH head
                # mean folds into e = exp(a_comb/NH) below)
                for hp in range(2):
                    for mi in range(MI):
                        ps = psp.tile([P, TS], F32, tag="ps")
                        nc.tensor.matmul(
                            out=ps[:],
                            lhsT=qTb_sb[hp * DH : (hp + 1) * DH, mc, mi * P : (mi + 1) * P],
                            rhs=kTb_sb[hp * DH : (hp + 1) * DH, mc, :],
                            start=True,
                            stop=True,
                        )
                        ex = wp.tile([P, TS], BF16, tag="ex")
                        se = wp.tile([P, 1], F32, tag="se")
                        nc.scalar.activation(
                            ex[:], ps[:], AF.Exp,
                            bias=0.0, scale=0.125, accum_out=se[:, 0:1],
                        )
                        r8 = wp.tile([P, 1], F32, tag="r8")
                        nc.vector.reciprocal(r8[:], se[:, 0:1])
                        if mc == 0 and hp == 0:
                            nc.vector.tensor_scalar_mul(
                                attn_sb[:, mi, :], ex[:], r8[:, 0:1]
                            )
                        else:
                            nc.vector.scalar_tensor_tensor(
                                out=attn_sb[:, mi, :],
                                in0=ex[:],
                                scalar=r8[:, 0:1],
                                in1=attn_sb[:, mi, :],
                                op0=ALU.mult,
                                op1=ALU.add,
                            )

            # ---- gate via the exp table (no sigmoid table switch):
            #      ez = exp(-(dec @ Wfcw.T + bfcw));  w = 1/(1+ez);
            #      s1 = 1-w = ez*w  (fp32 matmul for precision) ----
            for mi in range(MI):
                psg = psp.tile([P, 1], F32, tag="ps")
                for kc in range(KC):
                    nc.tensor.matmul(
                        out=psg[:],
                        lhsT=decT_sb[:, kc, mi * P : (mi + 1) * P],
                        rhs=wfcw_sb[:, kc : kc + 1],
                        start=(kc == 0),
                        stop=(kc == KC - 1),
                    )
                nc.scalar.activation(
                    ez_sb[:, mi : mi + 1], psg[:], AF.Exp,
                    bias=nbfcw_sb[:, 0:1], scale=-1.0,
                )
            nc.vector.tensor_scalar_add(t1_sb[:], ez_sb[:], 1.0)
            nc.vector.reciprocal(w_sb[:], t1_sb[:])
            nc.vector.tensor_tensor(
                out=s1_sb[:], in0=ez_sb[:], in1=w_sb[:], op=ALU.mult
            )

            p1_v = p1[:].rearrange("(mi p) v -> p mi v", p=P)
            out_v = out[:].rearrange("(mi p) v -> p mi v", p=P)

            # ---- epilogue per mi: attn_T via PE transpose, a_comb = attn@Dm,
            #      e = exp(a_comb/NH), denom, s2 = w/denom ----
            for mi in range(MI):
                for sc in range(SC):
                    pt = psp.tile([P, P], BF16, tag="ps")
                    nc.tensor.transpose(
                        out=pt[:],
                        in_=attn_sb[:, mi, sc * P : (sc + 1) * P],
                        identity=identb_sb[:],
                    )
                    nc.vector.tensor_copy(attnT_sb[:, sc, mi * P : (mi + 1) * P], pt[:])
                ps = psp.tile([P, TS], F32, tag="ps")
                for c in range(SC):
                    nc.tensor.matmul(
                        out=ps[:],
                        lhsT=attnT_sb[:, c, mi * P : (mi + 1) * P],
                        rhs=Dm_sb[:, c, :],
                        start=(c == 0),
                        stop=(c == SC - 1),
                    )
                nc.scalar.activation(
                    e_sb[:, mi, :], ps[:], AF.Exp, bias=0.0, scale=1.0 / NH
                )
                g = wp.tile([P, TS], F32, tag="g")
                nc.vector.scalar_tensor_tensor(
                    out=g[:],
                    in0=e_sb[:, mi, :],
                    scalar=-1.0,
                    in1=invcntrow_sb[:],
                    op0=ALU.add,
                    op1=ALU.mult,
                )
                nc.vector.tensor_reduce(sume_sb[:, mi : mi + 1], g[:], AX.X, ALU.add)
                nc.vector.tensor_scalar_add(
                    denom_sb[:, mi : mi + 1], sume_sb[:, mi : mi + 1], float(V)
                )
                nc.vector.reciprocal(rden_sb[:, mi : mi + 1], denom_sb[:, mi : mi + 1])
            nc.vector.tensor_tensor(
                out=s2_sb[:], in0=w_sb[:], in1=rden_sb[:], op=ALU.mult
            )

            # ---- fix columns early (their store leads the scalar ring):
            #      fix = s1*p1c + s2*e ----
            for mi in range(MI):
                t2 = wp.tile([P, TS], F32, tag="fix_t2")
                nc.vector.tensor_scalar_mul(t2[:], e_sb[:, mi, :], s2_sb[:, mi : mi + 1])
                nc.vector.scalar_tensor_tensor(
                    out=fix_sb[:, mi, :],
                    in0=p1c_sb[:, mi, :],
                    scalar=s1_sb[:, mi : mi + 1],
                    op0=ALU.mult,
                    in1=t2[:],
                    op1=ALU.add,
                )
            nc.scalar.dma_start(
                out=fixc[:].rearrange("(mi p) s -> p mi s", p=P), in_=fix_sb[:]
            )

            def blend_tile(mi, vt, path, defer_store=False):
                vs = slice(vt * VT, (vt + 1) * VT)
                pin = pinp.tile([P, VT], BF16, tag="pin")
                nc.sync.dma_start(out=pin[:], in_=p1_v[:, mi, vs])
                pout = poutp.tile([P, VT], BF16, tag="pout")
                if path == "act":
                    # one scalar-engine op, fp32 internal, single bf16 round
                    nc.scalar.activation(
                        pout[:], pin[:], AF.Identity,
                        bias=s2_sb[:, mi : mi + 1],
                        scale=s1_sb[:, mi : mi + 1],
                    )
                else:
                    # DVE pair with fp32 intermediate: also a single bf16 round
                    t = tmidp.tile([P, VT], F32, tag="tmid")
                    nc.vector.tensor_scalar_mul(
                        t[:], pin[:], s1_sb[:, mi : mi + 1]
                    )
                    nc.vector.tensor_scalar_add(
                        pout[:], t[:], s2_sb[:, mi : mi + 1]
                    )
                if not defer_store:
                    nc.scalar.dma_start(out=out_v[:, mi, vs], in_=pout[:])
                return pout

            # both s2 known: stream all 16 tiles, alternating engines.  The
            # last 4 stores are deferred to the sync ring (issued after every
            # load is enqueued) so the tail drains on both rings.
            tiles = [(m, v) for m in range(MI) for v in range(NVT)]
            deferred = []
            for i, (mi, vt) in enumerate(tiles):
                path = "act" if i % 2 == 0 else "dve"
                defer = i >= len(tiles) - 4
                pout = blend_tile(mi, vt, path, defer)
                if defer:
                    deferred.append((mi, vt, pout))
            for mi, vt, pout in deferred:
                vs = slice(vt * VT, (vt + 1) * VT)
                nc.sync.dma_start(out=out_v[:, mi, vs], in_=pout[:])

    nc.finalize()
    return nc


def _get_nc():
    global _NC_CACHE
    if _NC_CACHE is None:
        _NC_CACHE = build_nc()
    return _NC_CACHE


def kernel(**inputs) -> np.ndarray:
    dec = np.asarray(inputs["dec_output"], dtype=np.float32)  # [4, 512, 512]
    enc = np.asarray(inputs["enc_output"], dtype=np.float32)  # [4, 512, 512]
    src = np.asarray(inputs["src"]).astype(np.int32)  # [4, 512]
    p1 = np.asarray(inputs["p1"], dtype=np.float32)  # [4, 512, 32000]
    WfcQ = np.asarray(inputs["WfcQ"], dtype=np.float32)
    bfcQ = np.asarray(inputs["bfcQ"], dtype=np.float32)
    Wq = np.asarray(inputs["Wq"], dtype=np.float32)
    bq = np.asarray(inputs["bq"], dtype=np.float32)
    Wk = np.asarray(inputs["Wk"], dtype=np.float32)
    bk = np.asarray(inputs["bk"], dtype=np.float32)
    Wfcw = np.asarray(inputs["Wfcw"], dtype=np.float32)
    bfcw = np.asarray(inputs["bfcw"], dtype=np.float32)

    B, TQ, _ = dec.shape
    n_cores = 8

    import ml_dtypes

    bf16 = ml_dtypes.bfloat16
    # fold fcQ into the query projection (cq feeds nothing else)
    Wqc = Wq @ WfcQ
    bqc = Wq @ bfcQ + bq
    wqcb = np.ascontiguousarray(Wqc.T.astype(bf16))
    wkb = np.ascontiguousarray(Wk.T.astype(bf16))

    in_maps = []
    for core in range(n_cores):
        b, qh = core // 2, core % 2
        qs = slice(qh * TQH, (qh + 1) * TQH)
        p1_slab = p1[b, qs, :]
        # packed per-partition constants: [p, c] = x[c*128 + p]
        pk = np.zeros((P, PK), np.float32)
        pk[:, 0:4] = src[b].reshape(SC, P).T
        pk[:, 4:8] = bqc.reshape(KC, P).T
        pk[:, 8:12] = bk.reshape(KC, P).T
        pk[:, 12:16] = Wfcw[0].reshape(KC, P).T
        pk[:, 16] = -bfcw[0]  # negated: gate uses exp(-(z + bfcw))
        in_maps.append(
            {
                "decT": np.ascontiguousarray(dec[b].T[:, qs]),
                "decTb": np.ascontiguousarray(dec[b].T[:, qs].astype(bf16)),
                "encTb": np.ascontiguousarray(enc[b].T.astype(bf16)),
                "wqcb": wqcb,
                "wkb": wkb,
                "pk": pk,
                "p1": np.ascontiguousarray(p1_slab.astype(bf16)),
                "p1c": np.ascontiguousarray(p1_slab[:, src[b]].astype(bf16)),
            }
        )

    nc = _get_nc()
    res = run_bass_kernel_spmd(nc, in_maps, core_ids=list(range(n_cores)))
    global _LAST_RESULTS
    _LAST_RESULTS = res

    out = np.empty((B, TQ, V), dtype=np.float32)
    for core in range(n_cores):
        b, qh = core // 2, core % 2
        qs = slice(qh * TQH, (qh + 1) * TQH)
        out[b, qs, :] = res.results[core]["out"].astype(np.float32)
        # place the corrected source-token columns (duplicates carry
        # identical values, so overwrite order does not matter)
        out[b, qs, :][:, src[b]] = res.results[core]["fixc"]
    return out


# revision 40
# speedup vs baseline: 1.1355x; 1.0099x over previous
"""CopyDecoder Trainium2 kernel (nn_CopyDecoder_5274219840242).

Sharding: 8 cores = 4 batches x 2 query-halves (data parallel, no collectives).

Per core (b, q-slab of 256 rows):
  - attention: Q/K projections (fcQ folded into Wq on the host:
    Q = dec @ (Wq@WfcQ).T + (Wq@bfcQ + bq); computed transposed so the
    contraction dim lands on partitions; bf16 operands, fp32 accumulate),
    per-head softmax (logits bounded, so no max-subtraction), head mean.
  - duplicate-combining selection matrix Dm[s,s'] = [src_s == src_s'] built by
    compare-vs-transpose; a_comb = attn @ Dm gives each source position the
    full scatter-sum of its token; e = exp(a_comb/NH).
  - denom[q] = V + sum_s (e[q,s]-1)/cnt[s]  (softmax denominator over vocab,
    exploiting exp(0)=1 for vocab entries no source token maps to).
  - streaming blend over p1 in BF16 both directions (tolerance is 2e-2):
    out = (1-w)*p1 + w/denom.  Halves HBM traffic vs fp32 streaming, which
    is the roofline here (~99% DMA active in the fp32 baseline trace).
    The fused DVE tensor_scalar double-rounds (bf16 intermediate), which
    costs ~1.1e-2 rel err; instead each tile takes a single-rounding path:
    either one scalar-engine activation (out = Identity(p1*s1 + s2), fp32
    internal, per-partition scale/bias APs) or a DVE pair (mul to fp32
    intermediate, add to bf16).  Tiles are split between the two engines
    so neither becomes the bottleneck.
  - fix values for the <=512 source-token columns:
    fix[q,s] = (1-w)*p1[q,src_s] + (w/denom)*e[q,s]
    (p1 columns are host-gathered fp32 into an extra input; the host writes
    the fix columns into the final output during unshard).

Queue split (two HWDGE rings share 16 DMA engines, ~23.5GB/s each busy):
  - sync ring: Q-side weights (wqcb, decTb) first, then the pure p1 bf16
    load stream.
  - scalar ring: packed constants, K-side weights (wkb, encTb), decT,
    p1c, then all out-stores + fixc.
Weights ride ahead of the p1 stream on both rings so the attention chain
(which gates the first store via s2) starts ~5us in, not ~25us.  The
chain runs per q-partition-tile (mi) so the first blend stores start
while the second tile's softmax is still in flight.
"""

import sys

sys.path.insert(0, "/opt/trn_rl_repo")

import numpy as np

import concourse.bacc as bacc
import concourse.bass as bass
import concourse.mybir as mybir
import concourse.tile as tile
from concourse.bass_utils import run_bass_kernel_spmd
from concourse.masks import make_identity

P = 128
D = 512
TS = 512
TQH = 256  # q rows per core
V = 32000
NH = 8
DH = 64
KC = D // P  # 4 contraction chunks
MI = TQH // P  # 2 q partition tiles
SC = TS // P  # 4 source-position chunks
VT = 4000  # vocab columns per blend tile (8000B bf16 per partition row)
NVT = V // VT  # 8 vocab tiles per q partition tile

F32 = mybir.dt.float32
BF16 = mybir.dt.bfloat16
I32 = mybir.dt.int32
AF = mybir.ActivationFunctionType
ALU = mybir.AluOpType
AX = mybir.AxisListType

# packed per-partition constants layout (f32 columns):
#   [0:4) bqc   [4:8) bk   [8:12) wfcw   [12] -bfcw   [13] V-n_unique
PK = 14

_NC_CACHE = None
_LAST_RESULTS = None


def build_nc():
    nc = bacc.Bacc("TRN2", target_bir_lowering=False, debug=False)

    decT = nc.dram_tensor("decT", [D, TQH], F32, kind="ExternalInput")
    decTb = nc.dram_tensor("decTb", [D, TQH], BF16, kind="ExternalInput")
    encTb = nc.dram_tensor("encTb", [D, TS], BF16, kind="ExternalInput")
    wqcb = nc.dram_tensor("wqcb", [D, D], BF16, kind="ExternalInput")
    wkb = nc.dram_tensor("wkb", [D, D], BF16, kind="ExternalInput")
    pk = nc.dram_tensor("pk", [P, PK], F32, kind="ExternalInput")
    dmx = nc.dram_tensor("dmx", [P, SC * TS], BF16, kind="ExternalInput")
    invcr = nc.dram_tensor("invcr", [P, TS], BF16, kind="ExternalInput")
    p1 = nc.dram_tensor("p1", [TQH, V], BF16, kind="ExternalInput")
    p1c = nc.dram_tensor("p1c", [TQH, TS], BF16, kind="ExternalInput")
    out = nc.dram_tensor("out", [TQH, V], BF16, kind="ExternalOutput")
    fixc = nc.dram_tensor("fixc", [TQH, TS], F32, kind="ExternalOutput")

    with tile.TileContext(nc) as tc:
        with (
            tc.tile_pool(name="const", bufs=1) as cp,
            tc.tile_pool(name="work", bufs=6) as wp,
            tc.tile_pool(name="pin", bufs=8) as pinp,
            tc.tile_pool(name="pout", bufs=4) as poutp,
            tc.tile_pool(name="tmid", bufs=2) as tmidp,
            tc.tile_pool(name="ps", bufs=8, space="PSUM") as psp,
        ):
            # ---- persistent SBUF tiles ----
            decT_sb = cp.tile([P, KC, TQH], F32, tag="decT_sb")
            decTb_sb = cp.tile([P, KC, TQH], BF16, tag="decTb_sb")
            encTb_sb = cp.tile([P, KC, TS], BF16, tag="encTb_sb")
            wqcb_sb = cp.tile([P, KC, D], BF16, tag="wqcb_sb")
            wkb_sb = cp.tile([P, KC, D], BF16, tag="wkb_sb")
            pk_sb = cp.tile([P, PK], F32, tag="pk_sb")
            identb_sb = cp.tile([P, P], BF16, tag="identb_sb")
            invcr_sb = cp.tile([P, TS], BF16, tag="invcr_sb")
            Dm_sb = cp.tile([P, SC, TS], BF16, tag="Dm_sb")
            qTb_sb = cp.tile([P, KC, TQH], BF16, tag="qTb_sb")
            kTb_sb = cp.tile([P, KC, TS], BF16, tag="kTb_sb")
            attn_sb = cp.tile([P, MI, TS], BF16, tag="attn_sb")
            attnB_sb = cp.tile([P, MI, TS], BF16, tag="attnB_sb")
            attnT_sb = cp.tile([P, SC, TQH], BF16, tag="attnT_sb")
            e_sb = cp.tile([P, MI, TS], F32, tag="e_sb")
            p1c_sb = cp.tile([P, MI, TS], BF16, tag="p1c_sb")
            fix_sb = cp.tile([P, MI, TS], F32, tag="fix_sb")
            sume_sb = cp.tile([P, MI], F32, tag="sume_sb")
            denom_sb = cp.tile([P, MI], F32, tag="denom_sb")
            rden_sb = cp.tile([P, MI], F32, tag="rden_sb")
            w_sb = cp.tile([P, MI], F32, tag="w_sb")
            ez_sb = cp.tile([P, MI], F32, tag="ez_sb")
            t1_sb = cp.tile([P, MI], F32, tag="t1_sb")
            s1_sb = cp.tile([P, MI], F32, tag="s1_sb")
            s2_sb = cp.tile([P, MI], F32, tag="s2_sb")

            bqc_sb = pk_sb[:, 0:4]
            bk_sb = pk_sb[:, 4:8]
            wfcw_sb = pk_sb[:, 8:12]
            nbfcw_sb = pk_sb[:, 12:13]  # NEGATED gate bias (exp-form gate)
            vmu_sb = pk_sb[:, 13:14]  # V - n_unique(src)

            # ---- loads: Q-side operands lead the sync ring (ahead of the
            #      p1 stream); K-side operands + the rest lead the scalar
            #      ring (ahead of the out-stores).  First column-block of
            #      each weight goes separately so mc=0 matmuls start early.
            wqc_v = wqcb[:].rearrange("(c p) q -> p c q", p=P)
            wk_v = wkb[:].rearrange("(c p) q -> p c q", p=P)
            nc.sync.dma_start(
                out=decTb_sb[:], in_=decTb[:].rearrange("(c p) q -> p c q", p=P)
            )
            nc.sync.dma_start(out=wqcb_sb[:, :, 0:P], in_=wqc_v[:, :, 0:P])
            nc.sync.dma_start(out=wqcb_sb[:, :, P:D], in_=wqc_v[:, :, P:D])
            nc.sync.dma_start(
                out=decT_sb[:], in_=decT[:].rearrange("(c p) q -> p c q", p=P)
            )
            nc.scalar.dma_start(out=pk_sb[:], in_=pk[:])
            nc.scalar.dma_start(
                out=encTb_sb[:], in_=encTb[:].rearrange("(c p) q -> p c q", p=P)
            )
            nc.scalar.dma_start(out=wkb_sb[:, :, 0:P], in_=wk_v[:, :, 0:P])
            nc.scalar.dma_start(out=wkb_sb[:, :, P:D], in_=wk_v[:, :, P:D])
            nc.scalar.dma_start(
                out=Dm_sb[:], in_=dmx[:].rearrange("p (c s) -> p c s", c=SC)
            )
            nc.scalar.dma_start(out=invcr_sb[:], in_=invcr[:])
            nc.scalar.dma_start(
                out=p1c_sb[:], in_=p1c[:].rearrange("(mi p) s -> p mi s", p=P)
            )
            make_identity(nc, identb_sb[:])

            # pull the EXP activation table in off the critical path (the
            # scalar engine runs exclusively exps until the blend phase)
            junk = wp.tile([P, 1], F32, tag="junk")
            nc.scalar.activation(junk[:], pk_sb[:, 0:1], AF.Exp, bias=0.0, scale=1.0)

            # ---- per-chunk Q/K projections (bias-add on DVE, so the scalar
            #      engine stays on the exp table) interleaved with the scores
            #      + per-head softmax for the two heads living in that chunk:
            #      softmax pipelines with the projections and BOTH mi chains
            #      finish together ----
            for mc in range(KC):
                psq = psp.tile([P, TQH], F32, tag="ps")
                for kc in range(KC):
                    nc.tensor.matmul(
                        out=psq[:],
                        lhsT=wqcb_sb[:, kc, mc * P : (mc + 1) * P],
                        rhs=decTb_sb[:, kc, :],
                        start=(kc == 0),
                        stop=(kc == KC - 1),
                    )
                nc.vector.tensor_scalar_add(
                    qTb_sb[:, mc, :], psq[:], bqc_sb[:, mc : mc + 1]
                )
                psk = psp.tile([P, TS], F32, tag="ps")
                for kc in range(KC):
                    nc.tensor.matmul(
                        out=psk[:],
                        lhsT=wkb_sb[:, kc, mc * P : (mc + 1) * P],
                        rhs=encTb_sb[:, kc, :],
                        start=(kc == 0),
                        stop=(kc == KC - 1),
                    )
                nc.vector.tensor_scalar_add(
                    kTb_sb[:, mc, :], psk[:], bk_sb[:, mc : mc + 1]
                )
                # heads 2*mc and 2*mc+1 need only chunk mc of Q_T/K_T.
                # logits are ~N(0,1) so exp without max-subtraction is safe;
                # accumulate the sum of per-head softmaxes into TWO partial
                # chains per mi (halves the DVE dependency chain; combined
                # after the loop).  mi=0 first so its epilogue starts sooner.
                for mi in range(MI):
                    for hp in range(2):
                        tgt = attn_sb if hp == 0 else attnB_sb
                        ps = psp.tile([P, TS], F32, tag="ps")
                        nc.tensor.matmul(
                            out=ps[:],
                            lhsT=qTb_sb[hp * DH : (hp + 1) * DH, mc, mi * P : (mi + 1) * P],
                            rhs=kTb_sb[hp * DH : (hp + 1) * DH, mc, :],
                            start=True,
                            stop=True,
                        )
                        ex = wp.tile([P, TS], BF16, tag="ex")
                        se = wp.tile([P, 1], F32, tag="se")
                        nc.scalar.activation(
                            ex[:], ps[:], AF.Exp,
                            bias=0.0, scale=0.125, accum_out=se[:, 0:1],
                        )
                        r8 = wp.tile([P, 1], F32, tag="r8")
                        nc.vector.reciprocal(r8[:], se[:, 0:1])
                        if mc == 0:
                            nc.vector.tensor_scalar_mul(
                                tgt[:, mi, :], ex[:], r8[:, 0:1]
                            )
                        else:
                            nc.vector.scalar_tensor_tensor(
                                out=tgt[:, mi, :],
                                in0=ex[:],
                                scalar=r8[:, 0:1],
                                in1=tgt[:, mi, :],
                                op0=ALU.mult,
                                op1=ALU.add,
                            )

            # ---- gate via the exp table (no sigmoid table switch):
            #      ez = exp(-(dec @ Wfcw.T + bfcw));  w = 1/(1+ez);
            #      s1 = 1-w = ez*w  (fp32 matmul for precision) ----
            for mi in range(MI):
                psg = psp.tile([P, 1], F32, tag="ps")
                for kc in range(KC):
                    nc.tensor.matmul(
                        out=psg[:],
                        lhsT=decT_sb[:, kc, mi * P : (mi + 1) * P],
                        rhs=wfcw_sb[:, kc : kc + 1],
                        start=(kc == 0),
                        stop=(kc == KC - 1),
                    )
                nc.scalar.activation(
                    ez_sb[:, mi : mi + 1], psg[:], AF.Exp,
                    bias=nbfcw_sb[:, 0:1], scale=-1.0,
                )
            nc.vector.tensor_scalar_add(t1_sb[:], ez_sb[:], 1.0)
            nc.vector.reciprocal(w_sb[:], t1_sb[:])
            nc.vector.tensor_tensor(
                out=s1_sb[:], in0=ez_sb[:], in1=w_sb[:], op=ALU.mult
            )

            p1_v = p1[:].rearrange("(mi p) v -> p mi v", p=P)
            out_v = out[:].rearrange("(mi p) v -> p mi v", p=P)

            def epilogue(mi):
                # combine the two partial softmax sums, attn_T via PE
                # transpose, a_comb = attn@Dm, e = exp(a_comb/NH), denom
                # (= V-U + sum_s e*invcnt, U folded in host-side), s2
                nc.vector.tensor_tensor(
                    out=attn_sb[:, mi, :], in0=attn_sb[:, mi, :],
                    in1=attnB_sb[:, mi, :], op=ALU.add,
                )
                for sc in range(SC):
                    pt = psp.tile([P, P], BF16, tag="ps")
                    nc.tensor.transpose(
                        out=pt[:],
                        in_=attn_sb[:, mi, sc * P : (sc + 1) * P],
                        identity=identb_sb[:],
                    )
                    nc.vector.tensor_copy(attnT_sb[:, sc, mi * P : (mi + 1) * P], pt[:])
                ps = psp.tile([P, TS], F32, tag="ps")
                for c in range(SC):
                    nc.tensor.matmul(
                        out=ps[:],
                        lhsT=attnT_sb[:, c, mi * P : (mi + 1) * P],
                        rhs=Dm_sb[:, c, :],
                        start=(c == 0),
                        stop=(c == SC - 1),
                    )
                nc.scalar.activation(
                    e_sb[:, mi, :], ps[:], AF.Exp, bias=0.0, scale=1.0 / NH
                )
                g = wp.tile([P, TS], F32, tag="g")
                nc.vector.tensor_tensor(
                    out=g[:], in0=e_sb[:, mi, :], in1=invcr_sb[:], op=ALU.mult
                )
                nc.vector.tensor_reduce(sume_sb[:, mi : mi + 1], g[:], AX.X, ALU.add)
                nc.vector.tensor_scalar_add(
                    denom_sb[:, mi : mi + 1], sume_sb[:, mi : mi + 1],
                    vmu_sb[:, 0:1],
                )
                nc.vector.reciprocal(rden_sb[:, mi : mi + 1], denom_sb[:, mi : mi + 1])
                nc.vector.tensor_tensor(
                    out=s2_sb[:, mi : mi + 1], in0=w_sb[:, mi : mi + 1],
                    in1=rden_sb[:, mi : mi + 1], op=ALU.mult,
                )

            def blend_tile(mi, vt, path, defer_store=False):
                vs = slice(vt * VT, (vt + 1) * VT)
                pin = pinp.tile([P, VT], BF16, tag="pin")
                nc.sync.dma_start(out=pin[:], in_=p1_v[:, mi, vs])
                pout = poutp.tile([P, VT], BF16, tag="pout")
                if path == "act":
                    # one scalar-engine op, fp32 internal, single bf16 round
                    nc.scalar.activation(
                        pout[:], pin[:], AF.Identity,
                        bias=s2_sb[:, mi : mi + 1],
                        scale=s1_sb[:, mi : mi + 1],
                    )
                else:
                    # DVE pair with fp32 intermediate: also a single bf16 round
                    t = tmidp.tile([P, VT], F32, tag="tmid")
                    nc.vector.tensor_scalar_mul(
                        t[:], pin[:], s1_sb[:, mi : mi + 1]
                    )
                    nc.vector.tensor_scalar_add(
                        pout[:], t[:], s2_sb[:, mi : mi + 1]
                    )
                if not defer_store:
                    nc.scalar.dma_start(out=out_v[:, mi, vs], in_=pout[:])
                return pout

            # mi=0 epilogue -> two early blends -> mi=1 epilogue -> fix ->
            # remaining tiles alternating engines.  The last 4 stores are
            # deferred to the sync ring (issued after every load is
            # enqueued) so the tail drains on both rings.
            epilogue(0)
            blend_tile(0, 0, "act")
            blend_tile(0, 1, "dve")
            epilogue(1)

            # fix columns: fix = s1*p1c + s2*e (store rides the scalar ring
            # behind the first two blend stores)
            for mi in range(MI):
                t2 = wp.tile([P, TS], F32, tag="fix_t2")
                nc.vector.tensor_scalar_mul(t2[:], e_sb[:, mi, :], s2_sb[:, mi : mi + 1])
                nc.vector.scalar_tensor_tensor(
                    out=fix_sb[:, mi, :],
                    in0=p1c_sb[:, mi, :],
                    scalar=s1_sb[:, mi : mi + 1],
                    op0=ALU.mult,
                    in1=t2[:],
                    op1=ALU.add,
                )
            nc.scalar.dma_start(
                out=fixc[:].rearrange("(mi p) s -> p mi s", p=P), in_=fix_sb[:]
            )

            tiles = [(0, v) for v in range(2, NVT)] + [(1, v) for v in range(NVT)]
            deferred = []
            for i, (mi, vt) in enumerate(tiles):
                path = "act" if i % 2 == 0 else "dve"
                defer = i >= len(tiles) - 4
                pout = blend_tile(mi, vt, path, defer)
                if defer:
                    deferred.append((mi, vt, pout))
            for mi, vt, pout in deferred:
                vs = slice(vt * VT, (vt + 1) * VT)
                nc.sync.dma_start(out=out_v[:, mi, vs], in_=pout[:])

    nc.finalize()
    return nc


def _get_nc():
    global _NC_CACHE
    if _NC_CACHE is None:
        _NC_CACHE = build_nc()
    return _NC_CACHE


def kernel(**inputs) -> np.ndarray:
    dec = np.asarray(inputs["dec_output"], dtype=np.float32)  # [4, 512, 512]
    enc = np.asarray(inputs["enc_output"], dtype=np.float32)  # [4, 512, 512]
    src = np.asarray(inputs["src"]).astype(np.int32)  # [4, 512]
    p1 = np.asarray(inputs["p1"], dtype=np.float32)  # [4, 512, 32000]
    WfcQ = np.asarray(inputs["WfcQ"], dtype=np.float32)
    bfcQ = np.asarray(inputs["bfcQ"], dtype=np.float32)
    Wq = np.asarray(inputs["Wq"], dtype=np.float32)
    bq = np.asarray(inputs["bq"], dtype=np.float32)
    Wk = np.asarray(inputs["Wk"], dtype=np.float32)
    bk = np.asarray(inputs["bk"], dtype=np.float32)
    Wfcw = np.asarray(inputs["Wfcw"], dtype=np.float32)
    bfcw = np.asarray(inputs["bfcw"], dtype=np.float32)

    B, TQ, _ = dec.shape
    n_cores = 8

    import ml_dtypes

    bf16 = ml_dtypes.bfloat16
    # fold fcQ into the query projection (cq feeds nothing else)
    Wqc = Wq @ WfcQ
    bqc = Wq @ bfcQ + bq
    wqcb = np.ascontiguousarray(Wqc.T.astype(bf16))
    wkb = np.ascontiguousarray(Wk.T.astype(bf16))

    in_maps = []
    for core in range(n_cores):
        b, qh = core // 2, core % 2
        qs = slice(qh * TQH, (qh + 1) * TQH)
        p1_slab = p1[b, qs, :]
        # packed per-partition constants: [p, c] = x[c*128 + p]
        pk = np.zeros((P, PK), np.float32)
        pk[:, 0:4] = bqc.reshape(KC, P).T
        pk[:, 4:8] = bk.reshape(KC, P).T
        pk[:, 8:12] = Wfcw[0].reshape(KC, P).T
        pk[:, 12] = -bfcw[0]  # negated: gate uses exp(-(z + bfcw))
        pk[:, 13] = float(V - np.unique(src[b]).size)
        # duplicate-combining selection matrix + reciprocal counts
        eq = (src[b][:, None] == src[b][None, :])  # [TS, TS]
        dmx = np.ascontiguousarray(
            eq.reshape(SC, P, TS).transpose(1, 0, 2).reshape(P, SC * TS).astype(bf16)
        )
        invc = (1.0 / eq.sum(1)).astype(np.float32)  # [TS]
        invcr = np.ascontiguousarray(
            np.broadcast_to(invc, (P, TS)).astype(bf16)
        )
        in_maps.append(
            {
                "decT": np.ascontiguousarray(dec[b].T[:, qs]),
                "decTb": np.ascontiguousarray(dec[b].T[:, qs].astype(bf16)),
                "encTb": np.ascontiguousarray(enc[b].T.astype(bf16)),
                "wqcb": wqcb,
                "wkb": wkb,
                "pk": pk,
                "dmx": dmx,
                "invcr": invcr,
                "p1": np.ascontiguousarray(p1_slab.astype(bf16)),
                "p1c": np.ascontiguousarray(p1_slab[:, src[b]].astype(bf16)),
            }
        )

    nc = _get_nc()
    res = run_bass_kernel_spmd(nc, in_maps, core_ids=list(range(n_cores)))
    global _LAST_RESULTS
    _LAST_RESULTS = res

    out = np.empty((B, TQ, V), dtype=np.float32)
    for core in range(n_cores):
        b, qh = core // 2, core % 2
        qs = slice(qh * TQH, (qh + 1) * TQH)
        out[b, qs, :] = res.results[core]["out"].astype(np.float32)
        # place the corrected source-token columns (duplicates carry
        # identical values, so overwrite order does not matter)
        out[b, qs, :][:, src[b]] = res.results[core]["fixc"]
    return out
